# revision 17
# baseline (speedup 1.0000x reference)
"""Trainium2 kernel for nn_IteratedLinearNet: y = x @ (W.T)^60.

Strategy (8 NeuronCores, single SPMD launch):
  - track T_k = W^k; 60 = 12 + 24 + 24, so y = ((x @ A12) @ A24) @ A24
    with A_k = (W.T)^k = T_k.T — a THREE-STAGE apply whose stationaries
    come straight from raw slab AllGathers (no transpose needed), and
    the last two stages share one stationary
  - chain: only 6 products using THREE stationary matrices {W, T3, T12}:
      T2=T1*W, T3=T2*W              (stationary: W, DRAM input)
      T6=T3*T3, T9=T6*T3, T12=T9*T3 (stationary: gathered T3)
      T24=T12*T12                   (stationary: gathered T12)
  - 3 AllGathers total: T3 / T12 land in the chain stationary via XBAR
    DMA-transpose (T12's raw blocks also feed the stage-1 stationary
    A12 directly); T24 raw feeds A24; the T24 gather is fully hidden
    behind the stage-1 apply
  - a tiny warmup AllGather runs during the prologue to absorb the
    collective stack's ~50us first-use setup
  - row-sharded slabs: core j holds U_k = (T_k[Rj,:]).T [2048, 256];
    each product is U_next = G.T @ U with gathered G native as lhsT;
    every step's rhs is the previous step's output (3-slab ring)
  - steps that follow a stationary load run their matmuls K-MAJOR with
    8 open PSUM groups so MM consumption tracks stationary-row arrival
  - plain DMAs alternate between the SP HWDGE queue and the gpsimd
    software DGE (the Activation HWDGE queue corrupts data on this
    runtime); XBAR transposes are HWDGE-only so they stay on SP
  - float16 at full PE rate with power-of-2 rescaling per product/stage
    (exact, folded into the PSUM evacuation); PSUM accumulates fp32;
    host multiplies the fp32 output by 2^-50.  Simulated rel err 2.3e-3
    against the fp32 reference (gate 2e-2).

Self-contained: builds/compiles on first call and caches the module.
"""

import numpy as np

_G = 2048
_B = 4096
_NC = 8
_SW = _G // _NC  # 256 slab width (rows per core)
_BW = _B // _NC  # 512 batch rows per core
_KT = _G // 128  # 16

# (power, stat: 0=W/g12(stat0) 1=g3(stat1), shift, export, kmajor)
# slab ring: step i reads slab[i%3], writes slab[(i+1)%3]
_STEPS = [
    (2, 0, 6, False, True),
    (3, 0, 1, True, False),  # -> g3 (XBAR -> stat1)
    (6, 1, -5, False, True),
    (9, 1, -4, False, False),
    (12, 1, -5, True, False),  # -> g12 (XBAR -> stat0, raw -> A12/stat1)
    (24, 0, -5, True, True),  # -> g24 (raw -> A24/stat0)
]
_ZS1 = -6
_ZS2 = -4
_TOTAL_SHIFT = 50  # sh12(14) + zs1 + sh24(23) + zs2 + sh24(23)

_cache = {}


def _build():
    from contextlib import ExitStack

    import concourse.tile as tile
    from concourse import bacc, mybir

    F16 = mybir.dt.float16
    F32 = mybir.dt.float32
    G, KT, SW, BW = _G, _KT, _SW, _BW

    nc = bacc.Bacc(None, target_bir_lowering=False, num_devices=_NC)
    w16 = nc.declare_dram_parameter("w16", [G, G], F16, isOutput=False)
    u1 = nc.declare_dram_parameter("u1", [G, SW], F16, isOutput=False)
    xt = nc.declare_dram_parameter("xt", [G, BW], F16, isOutput=False)
    ytj = nc.declare_dram_parameter("ytj", [G, BW], F32, isOutput=True)

    rg = [list(range(_NC))]

    with ExitStack() as ctx:
        tc = ctx.enter_context(tile.TileContext(nc))
        statp = ctx.enter_context(tc.tile_pool(name="statp", bufs=1))
        slabs = ctx.enter_context(tc.tile_pool(name="slabs", bufs=1))
        ypool = ctx.enter_context(tc.tile_pool(name="ypool", bufs=2))
        mmps = ctx.enter_context(tc.tile_pool(name="mmps", bufs=8, space="PSUM"))
        dram = ctx.enter_context(tc.tile_pool(name="dram", bufs=1, space="DRAM"))

        stat = [
            statp.tile([128, KT, G], F16, name=f"stat{i}", tag=f"stat{i}")
            for i in range(2)
        ]
        slab = [
            slabs.tile([128, KT, SW], F16, name=f"slab{i}", tag=f"slab{i}")
            for i in range(3)
        ]
        xts = slabs.tile([128, KT, BW], F16, name="xts", tag="xts")
        zt = slabs.tile([128, KT, BW], F16, name="zt", tag="zt")

        q = [nc.sync, nc.gpsimd]  # SP HWDGE + software DGE

        # warmup AllGather: absorbs the collective first-use penalty
        # while the prologue DMAs run; output is never consumed
        warm_in = dram.tile([128, 64], F16, name="warm_in", tag="warm_in")
        nc.sync.dma_start(warm_in[:, :], u1[0:128, 0:64])
        warm_out = dram.tile(
            [128 * _NC, 64], F16, name="warm_out", tag="warm_out",
            addr_space="Shared",
        )
        nc.gpsimd.collective_compute(
            "AllGather",
            mybir.AluOpType.bypass,
            replica_groups=rg,
            ins=[warm_in.opt()],
            outs=[warm_out.opt()],
        )

        # prologue: first slab, then W rows in k order (k-major s1 MMs
        # start consuming after the first row lands)
        for k in range(KT):
            q[k % 2].dma_start(slab[0][:, k, :], u1[128 * k : 128 * (k + 1), :])
        for k in range(KT):
            q[k % 2].dma_start(stat[0][:, k, :], w16[128 * k : 128 * (k + 1), :])

        def half_mms(st, rhs, out, scale, h, kmajor):
            mr = range(8 * h, 8 * h + 8)
            pss = {
                m: mmps.tile([128, BW], F32, name="ps", tag="ps")[:, 0:SW]
                for m in mr
            }
            order = (
                [(k, m) for k in range(KT) for m in mr]
                if kmajor
                else [(k, m) for m in mr for k in range(KT)]
            )
            for k, m in order:
                nc.tensor.matmul(
                    pss[m],
                    st[:, k, 128 * m : 128 * (m + 1)],
                    rhs[:, k, :],
                    start=(k == 0),
                    stop=(k == KT - 1),
                )
            for m in mr:
                nc.vector.tensor_scalar_mul(out[:, m, :], pss[m], scale)

        def apply_chunks(ast, g):
            # ast[:, k, SW*j:...] <- raw gathered blocks, j-major
            for j in range(_NC):
                for k in range(KT):
                    q[k % 2].dma_start(
                        ast[:, k, SW * j : SW * (j + 1)],
                        g[G * j + 128 * k : G * j + 128 * (k + 1), :],
                    )

        def apply_mms(ast, rhs_t, out_evac):
            for m in range(KT):
                psY = mmps.tile([128, BW], F32, name="ps", tag="ps")
                for k in range(KT):
                    nc.tensor.matmul(
                        psY[:],
                        ast[:, k, 128 * m : 128 * (m + 1)],
                        rhs_t[:, k, :],
                        start=(k == 0),
                        stop=(k == KT - 1),
                    )
                out_evac(m, psY)

        ag_raw = {}
        for si, (pw, sb, shift, export, kmajor) in enumerate(_STEPS):
            st = stat[sb]
            rhs = slab[si % 3]
            out = slab[(si + 1) % 3]
            scale = float(2.0**shift)
            for h in range(2):
                half_mms(st, rhs, out, scale, h, kmajor and h == 0)
            if export:
                u_in = dram.tile([G, SW], F16, name=f"uin{pw}", tag=f"uin{pw}")
                for k in range(KT):
                    q[k % 2].dma_start(
                        u_in[128 * k : 128 * (k + 1), :], out[:, k, :]
                    )
                g = dram.tile(
                    [_NC * G, SW], F16, name=f"g{pw}", tag=f"g{pw}",
                    addr_space="Shared",
                )
                nc.gpsimd.collective_compute(
                    "AllGather",
                    mybir.AluOpType.bypass,
                    replica_groups=rg,
                    ins=[u_in.opt()],
                    outs=[g.opt()],
                )
                ag_raw[pw] = g
                if pw in (3, 12):
                    # XBAR-transpose gathered raw slabs into the chain
                    # stationary: stat[:, k, :] = T[128k+p, :]
                    nst = stat[1 - sb]
                    for k in range(KT):
                        j = k // 2
                        c0 = 128 * (k % 2)
                        nc.sync.dma_start(
                            nst[:, k, :],
                            g[G * j : G * (j + 1), c0 : c0 + 128],
                            transpose=True,
                        )
                if pw == 12:
                    # stage-1 A12 chunks must be queued before T24's
                    # export DMAs (which wait on T24) or they'd block
                    # the stage-1 apply behind a head-of-line stall
                    apply_chunks(stat[1], g)  # g3 dead after T12
            if si == 1:
                # stream x.T during the long T6/T9/T12 burst
                for k in range(KT):
                    q[k % 2].dma_start(
                        xts[:, k, :], xt[128 * k : 128 * (k + 1), :]
                    )

        # stage 1: z1 = x @ A12 -> zt; runs while AG(T24) is in flight
        zs1 = float(2.0**_ZS1)
        apply_mms(
            stat[1], xts,
            lambda m, psY: nc.vector.tensor_scalar_mul(zt[:, m, :], psY[:], zs1),
        )
        # stage 2: z2 = z1 @ A24 -> xts (dead after stage 1, reused)
        apply_chunks(stat[0], ag_raw[24])  # g12-native dead after T24
        zs2 = float(2.0**_ZS2)
        apply_mms(
            stat[0], zt,
            lambda m, psY: nc.vector.tensor_scalar_mul(xts[:, m, :], psY[:], zs2),
        )

        # stage 3: y = z2 @ A24 (same stationary, no reload)
        def y_evac(m, psY):
            ystage = ypool.tile([128, BW], F32, name="ystage", tag="ystage")
            nc.vector.tensor_copy(ystage[:], psY[:])
            q[m % 2].dma_start(ytj[128 * m : 128 * (m + 1), :], ystage[:])

        apply_mms(stat[0], xts, y_evac)
    nc.compile()
    return nc


def kernel(x, W):
    from concourse.bass_utils import run_bass_kernel_spmd

    if "nc" not in _cache:
        _cache["nc"] = _build()
    nc = _cache["nc"]

    W = np.asarray(W, dtype=np.float32)
    x = np.asarray(x, dtype=np.float32)
    w16_np = np.ascontiguousarray(W.astype(np.float16))
    xt_np = x.T.astype(np.float16)
    in_maps = [
        {
            "w16": w16_np,
            "u1": np.ascontiguousarray(w16_np[_SW * j : _SW * (j + 1), :].T),
            "xt": np.ascontiguousarray(xt_np[:, _BW * j : _BW * (j + 1)]),
        }
        for j in range(_NC)
    ]
    res = run_bass_kernel_spmd(nc, in_maps, core_ids=list(range(_NC)))
    _cache["last_exec_time_ns"] = res.exec_time_ns
    _cache["last_results"] = res
    unscale = np.float32(2.0**-_TOTAL_SHIFT)
    y = np.concatenate(
        [res.results[j]["ytj"].T * unscale for j in range(_NC)], axis=0
    ).astype(np.float32)
    return y


# revision 18
# speedup vs baseline: 1.1132x; 1.1132x over previous
"""Trainium2 kernel for nn_IteratedLinearNet: y = x @ (W.T)^60.

Strategy (8 NeuronCores, single SPMD launch):
  - track T_k = W^k; 60 = 12 + 24 + 24, so y = ((x @ A12) @ A24) @ A24
    with A_k = (W.T)^k = T_k.T — a THREE-STAGE apply whose stationaries
    come straight from raw slab AllGathers (no transpose), and the last
    two stages share one stationary
  - chain: only 6 products using THREE stationary matrices {W, T3, T12}:
      T2=T1*W, T3=T2*W              (stationary: W, DRAM input)
      T6=T3*T3, T9=T6*T3, T12=T9*T3 (stationary: gathered T3)
      T24=T12*T12                   (stationary: gathered T12)
  - chain stationaries (T3, T12) are PE-transposed locally (32 128x128
    transposes, ~9us, hidden) and AllGathered in NATIVE orientation, so
    stationary loads are plain [128, 2048] row DMAs at full DMA rate
    (the XBAR DMA-transpose path moves 256B packets at ~60 GB/s — 5x
    slower — and the gpsimd software DGE is ~27 GB/s; neither touches
    the critical path here)
  - T12 is additionally AllGathered raw (for the A12 stage-1 stationary)
    and T24 raw only; the raw gathers and the T24 gather hide behind the
    T24 product and the stage-1 apply
  - a tiny warmup AllGather absorbs the collective stack's ~50us
    first-use setup during the prologue
  - products that follow a stationary load run their matmuls K-MAJOR
    (6 open PSUM groups) so MM consumption tracks row arrival
  - float16 at full PE rate with power-of-2 rescaling per product/stage
    (exact, folded into the PSUM evacuation); PSUM accumulates fp32;
    host multiplies the fp32 output by 2^-50.  Simulated rel err 2.3e-3
    against the fp32 reference (gate 2e-2).

Self-contained: builds/compiles on first call and caches the module.
"""

import numpy as np

_G = 2048
_B = 4096
_NC = 8
_SW = _G // _NC  # 256 slab width (rows per core)
_BW = _B // _NC  # 512 batch rows per core
_KT = _G // 128  # 16

# (power, stat: 0=W/g12(stat0) 1=g3(stat1), shift, nat_export, raw_export,
#  kmajor) — slab ring: step i reads slab[i%3], writes slab[(i+1)%3]
_STEPS = [
    (2, 0, 6, False, False, True),
    (3, 0, 1, True, False, False),  # nat g3 -> stat1
    (6, 1, -5, False, False, True),
    (9, 1, -4, False, False, False),
    (12, 1, -5, True, True, False),  # nat g12 -> stat0; raw -> A12/stat1
    (24, 0, -5, False, True, True),  # raw g24 -> A24/stat0
]
_ZS1 = -6
_ZS2 = -4
_TOTAL_SHIFT = 50  # sh12(14) + zs1 + sh24(23) + zs2 + sh24(23)

_cache = {}


def _build():
    from contextlib import ExitStack

    import concourse.tile as tile
    from concourse import bacc, masks, mybir

    F16 = mybir.dt.float16
    F32 = mybir.dt.float32
    G, KT, SW, BW = _G, _KT, _SW, _BW

    nc = bacc.Bacc(None, target_bir_lowering=False, num_devices=_NC)
    w16 = nc.declare_dram_parameter("w16", [G, G], F16, isOutput=False)
    u1 = nc.declare_dram_parameter("u1", [G, SW], F16, isOutput=False)
    xt = nc.declare_dram_parameter("xt", [G, BW], F16, isOutput=False)
    ytj = nc.declare_dram_parameter("ytj", [G, BW], F32, isOutput=True)

    rg = [list(range(_NC))]

    with ExitStack() as ctx:
        tc = ctx.enter_context(tile.TileContext(nc))
        statp = ctx.enter_context(tc.tile_pool(name="statp", bufs=1))
        slabs = ctx.enter_context(tc.tile_pool(name="slabs", bufs=1))
        ypool = ctx.enter_context(tc.tile_pool(name="ypool", bufs=2))
        mmps = ctx.enter_context(tc.tile_pool(name="mmps", bufs=6, space="PSUM"))
        tps = ctx.enter_context(tc.tile_pool(name="tps", bufs=2, space="PSUM"))
        dram = ctx.enter_context(tc.tile_pool(name="dram", bufs=1, space="DRAM"))

        stat = [
            statp.tile([128, KT, G], F16, name=f"stat{i}", tag=f"stat{i}")
            for i in range(2)
        ]
        slab = [
            slabs.tile([128, KT, SW], F16, name=f"slab{i}", tag=f"slab{i}")
            for i in range(3)
        ]
        xts = slabs.tile([128, KT, BW], F16, name="xts", tag="xts")
        zt = slabs.tile([128, KT, BW], F16, name="zt", tag="zt")
        v_sb = slabs.tile([128, 2, G], F16, name="v_sb", tag="v_sb")
        ident32 = slabs.tile([128, 128], F32, name="ident32", tag="ident32")
        masks.make_identity(nc, ident32[:])
        ident = slabs.tile([128, 128], F16, name="ident", tag="ident")
        nc.vector.tensor_copy(ident[:], ident32[:])

        # queues: SP HWDGE for everything ordering-critical; Activation
        # HWDGE only for stationary-row/chunk loads (experiment — it
        # corrupted when carrying XBAR+everything, plain loads may be
        # fine); gpsimd SWDGE (slow, ~27 GB/s) only for xts prefetch
        qrow = [nc.sync, nc.scalar]

        # warmup AllGather: absorbs the collective first-use penalty
        warm_in = dram.tile([128, 64], F16, name="warm_in", tag="warm_in")
        nc.sync.dma_start(warm_in[:, :], u1[0:128, 0:64])
        warm_out = dram.tile(
            [128 * _NC, 64], F16, name="warm_out", tag="warm_out",
            addr_space="Shared",
        )
        nc.gpsimd.collective_compute(
            "AllGather",
            mybir.AluOpType.bypass,
            replica_groups=rg,
            ins=[warm_in.opt()],
            outs=[warm_out.opt()],
        )

        # prologue: W rows k-ordered on the fast queue (paces s1 k-major),
        # first slab + x.T on the others
        for k in range(KT):
            nc.sync.dma_start(stat[0][:, k, :], w16[128 * k : 128 * (k + 1), :])
        for k in range(KT):
            nc.scalar.dma_start(slab[0][:, k, :], u1[128 * k : 128 * (k + 1), :])
        for k in range(KT):
            nc.gpsimd.dma_start(xts[:, k, :], xt[128 * k : 128 * (k + 1), :])

        def step_mms(st, rhs, out, scale, kmajor):
            for mr in (range(0, 6), range(6, 12), range(12, 16)):
                pss = {
                    m: mmps.tile([128, BW], F32, name="ps", tag="ps")[:, 0:SW]
                    for m in mr
                }
                order = (
                    [(k, m) for k in range(KT) for m in mr]
                    if kmajor and mr.start == 0
                    else [(k, m) for m in mr for k in range(KT)]
                )
                for k, m in order:
                    nc.tensor.matmul(
                        pss[m],
                        st[:, k, 128 * m : 128 * (m + 1)],
                        rhs[:, k, :],
                        start=(k == 0),
                        stop=(k == KT - 1),
                    )
                for m in mr:
                    nc.vector.tensor_scalar_mul(out[:, m, :], pss[m], scale)

        def ag(name, src_dram, out_rows):
            g = dram.tile(
                [out_rows, src_dram.shape[-1]], F16, name=name, tag=name,
                addr_space="Shared",
            )
            nc.gpsimd.collective_compute(
                "AllGather",
                mybir.AluOpType.bypass,
                replica_groups=rg,
                ins=[src_dram.opt()],
                outs=[g.opt()],
            )
            return g

        def apply_chunks(ast, g):
            for j in range(_NC):
                for k in range(KT):
                    qrow[k % 2].dma_start(
                        ast[:, k, SW * j : SW * (j + 1)],
                        g[G * j + 128 * k : G * j + 128 * (k + 1), :],
                    )

        def apply_mms(ast, rhs_t, out_evac):
            for m in range(KT):
                psY = mmps.tile([128, BW], F32, name="ps", tag="ps")
                for k in range(KT):
                    nc.tensor.matmul(
                        psY[:],
                        ast[:, k, 128 * m : 128 * (m + 1)],
                        rhs_t[:, k, :],
                        start=(k == 0),
                        stop=(k == KT - 1),
                    )
                out_evac(m, psY)

        ag_raw = {}
        for si, (pw, sb, shift, nat_exp, raw_exp, kmajor) in enumerate(_STEPS):
            st = stat[sb]
            rhs = slab[si % 3]
            out = slab[(si + 1) % 3]
            step_mms(st, rhs, out, float(2.0**shift), kmajor)
            if nat_exp:
                # PE-transpose own slab -> native row slab, gather, and
                # load the next stationary as plain full-speed row DMAs
                for m in range(KT):
                    for a in range(2):
                        psT = tps.tile([128, 128], F16, name="psT", tag="psT")
                        nc.tensor.transpose(
                            psT[:], out[:, m, 128 * a : 128 * (a + 1)], ident[:]
                        )
                        nc.vector.tensor_copy(
                            v_sb[:, a, 128 * m : 128 * (m + 1)], psT[:]
                        )
                v_in = dram.tile([SW, G], F16, name=f"vin{pw}", tag=f"vin{pw}")
                for a in range(2):
                    nc.sync.dma_start(
                        v_in[128 * a : 128 * (a + 1), :], v_sb[:, a, :]
                    )
                gnat = ag(f"gn{pw}", v_in, SW * _NC)
                nst = stat[1 - sb]
                for k in range(KT):
                    qrow[k % 2].dma_start(
                        nst[:, k, :], gnat[128 * k : 128 * (k + 1), :]
                    )
            if raw_exp:
                u_in = dram.tile([G, SW], F16, name=f"uin{pw}", tag=f"uin{pw}")
                for k in range(KT):
                    nc.sync.dma_start(
                        u_in[128 * k : 128 * (k + 1), :], out[:, k, :]
                    )
                ag_raw[pw] = ag(f"gr{pw}", u_in, G * _NC)
                if pw == 12:
                    # stage-1 A12 chunks queue before T24's export DMAs
                    # (which wait on T24) to avoid head-of-line stalls
                    apply_chunks(stat[1], ag_raw[12])  # g3 dead after T12

        # stage 1: z1 = x @ A12 -> zt; runs while AG(T24 raw) is in flight
        zs1 = float(2.0**_ZS1)
        apply_mms(
            stat[1], xts,
            lambda m, psY: nc.vector.tensor_scalar_mul(zt[:, m, :], psY[:], zs1),
        )
        # stage 2: z2 = z1 @ A24 -> xts (dead after stage 1, reused)
        apply_chunks(stat[0], ag_raw[24])  # g12-native dead after T24
        zs2 = float(2.0**_ZS2)
        apply_mms(
            stat[0], zt,
            lambda m, psY: nc.vector.tensor_scalar_mul(xts[:, m, :], psY[:], zs2),
        )

        # stage 3: y = z2 @ A24 (same stationary, no reload)
        def y_evac(m, psY):
            ystage = ypool.tile([128, BW], F32, name="ystage", tag="ystage")
            nc.vector.tensor_copy(ystage[:], psY[:])
            nc.sync.dma_start(ytj[128 * m : 128 * (m + 1), :], ystage[:])

        apply_mms(stat[0], xts, y_evac)
    nc.compile()
    return nc


def kernel(x, W):
    from concourse.bass_utils import run_bass_kernel_spmd

    if "nc" not in _cache:
        _cache["nc"] = _build()
    nc = _cache["nc"]

    W = np.asarray(W, dtype=np.float32)
    x = np.asarray(x, dtype=np.float32)
    w16_np = np.ascontiguousarray(W.astype(np.float16))
    xt_np = x.T.astype(np.float16)
    in_maps = [
        {
            "w16": w16_np,
            "u1": np.ascontiguousarray(w16_np[_SW * j : _SW * (j + 1), :].T),
            "xt": np.ascontiguousarray(xt_np[:, _BW * j : _BW * (j + 1)]),
        }
        for j in range(_NC)
    ]
    res = run_bass_kernel_spmd(nc, in_maps, core_ids=list(range(_NC)))
    _cache["last_exec_time_ns"] = res.exec_time_ns
    _cache["last_results"] = res
    unscale = np.float32(2.0**-_TOTAL_SHIFT)
    y = np.concatenate(
        [res.results[j]["ytj"].T * unscale for j in range(_NC)], axis=0
    ).astype(np.float32)
    return y
